# revision 12
# baseline (speedup 1.0000x reference)
"""Distributed causal multi-head attention for Trainium2 (8 NeuronCores).

Problem: x:(1,4096,2048), W_{K,Q,V}:(16,128,2048), W_O:(2048,2048)
  k/q/v = einsum('ihd,bpd->biph'), scores=q@k^T causal-masked softmax, z=attn@v,
  out = einsum('df,bqf->bqd', W_O, z_flat)

Sharding: tensor-parallel over heads. Core c owns heads {2c, 2c+1}:
  - computes Q^T/K^T ([dh,seq] layout) and V ([seq,dh]) for its 2 heads
  - attention entirely on-core, scores kept transposed [key_pos, q] so the
    softmax denominator is a ones-matmul and AV needs no transposes
  - normalized z^T [f=head*128+h, q] is AllToAll'd so core c ends up with
    z^T[all 2048 f, its 512 q rows]; it then computes out rows [512c:512c+512]
    with the full W_O^T. Host concatenates the 8 row-blocks.

All matmul operands are bf16 (1 cycle/row on PE vs 4 for fp32); accumulation
fp32 in PSUM. exp/softmax stats in fp32.
"""

import os
import sys

import numpy as np

for _p in ("/opt/trn_rl_repo", "/root/.axon_site/_ro/trn_rl_repo"):
    if os.path.isdir(_p) and _p not in sys.path:
        sys.path.insert(0, _p)

import ml_dtypes  # noqa: E402

import concourse.bass as bass  # noqa: E402
import concourse.mybir as mybir  # noqa: E402
import concourse.tile as tile  # noqa: E402
from concourse import bacc  # noqa: E402
from concourse.bass_utils import run_bass_kernel_spmd  # noqa: E402

P = 128          # partitions
S = 4096         # sequence
D = 2048         # d_model
DH = 128         # head dim
NCORES = 8
HPC = 2          # heads per core
QT = 512         # q tile (matmul free dim)
NQT = S // QT    # 8 q tiles
SQ = S // NCORES # 512 out rows per core
NDT = D // P     # 16 d-model tiles
F = HPC * DH     # 256 local z rows
BF = mybir.dt.bfloat16
F32 = mybir.dt.float32
INV_SQRT_DH = 1.0 / float(np.sqrt(DH))

_CACHED_NC = None


def build():
    nc = bacc.Bacc("TRN2", target_bir_lowering=False, debug=False,
                   num_devices=NCORES)

    xT = nc.dram_tensor("xT", [D, S], BF, kind="ExternalInput").ap()
    wqT = nc.dram_tensor("wqT", [D, F], BF, kind="ExternalInput").ap()
    wkT = nc.dram_tensor("wkT", [D, F], BF, kind="ExternalInput").ap()
    wvT = nc.dram_tensor("wvT", [D, F], BF, kind="ExternalInput").ap()
    woT = nc.dram_tensor("woT", [D, D], BF, kind="ExternalInput").ap()
    out = nc.dram_tensor("out", [SQ, D], F32, kind="ExternalOutput").ap()
    # AllToAll bounce buffers. Row r = dest-chunk r//256, f-row r%256.
    z_send = nc.dram_tensor("z_send", [S // 2, QT], BF).ap()
    z_recv = nc.dram_tensor("z_recv", [S // 2, QT], BF).ap()

    xT_r = xT.rearrange("(o i) s -> i o s", i=P)       # [128,16,4096]
    wqT_r = wqT.rearrange("(o i) f -> i o f", i=P)     # [128,16,256]
    wkT_r = wkT.rearrange("(o i) f -> i o f", i=P)
    wvT_r = wvT.rearrange("(o i) f -> i o f", i=P)
    woT_r = woT.rearrange("(o i) d -> i o d", i=P)     # [128,16,2048]
    zr_r = z_recv.rearrange("(o i) q -> i o q", i=P)   # [128,16,512]

    with tile.TileContext(nc) as tc:
        _body(tc, nc, xT_r, wqT_r, wkT_r, wvT_r, woT_r, z_send, z_recv, zr_r,
              out)

    nc.compile()
    return nc


def _body(tc, nc, xT_r, wqT_r, wkT_r, wvT_r, woT_r, z_send, z_recv, zr_r,
          out):
    mult = mybir.AluOpType.mult
    Exp = mybir.ActivationFunctionType.Exp

    with (
        tc.tile_pool(name="const", bufs=1) as const,
        tc.tile_pool(name="w", bufs=1) as wpool,
        tc.tile_pool(name="big", bufs=1) as big,
        tc.tile_pool(name="xt", bufs=2) as xpool,
        tc.tile_pool(name="pt", bufs=4) as ptpool,
        tc.tile_pool(name="small", bufs=3) as small,
        tc.tile_pool(name="zt", bufs=2) as ztpool,
        tc.tile_pool(name="osb", bufs=3) as opool,
        tc.tile_pool(name="psum512", bufs=4, space="PSUM") as psum512,
        tc.tile_pool(name="psum1024", bufs=2, space="PSUM") as psum1024,
    ):
        # ---- constants -------------------------------------------------
        ones_sb = const.tile([P, 1], BF)
        nc.gpsimd.memset(ones_sb[:], 1.0)
        # sliding causal mask: mask[p, j] = 1.0 if j - p - 384 >= 0 else 0
        # slice [384-off : 896-off] gives keep iff (qlocal - p - off) >= 0
        mask_sb = const.tile([P, 384 + QT], BF)
        nc.gpsimd.memset(mask_sb[:], 1.0)
        nc.gpsimd.affine_select(
            out=mask_sb[:], in_=mask_sb[:],
            compare_op=mybir.AluOpType.is_ge,
            fill=0.0, base=-384,
            pattern=[[1, 384 + QT]], channel_multiplier=-1,
        )

        # ---- persistent SBUF tensors ----------------------------------
        wq_sb = wpool.tile([P, NDT, F], BF)
        wk_sb = wpool.tile([P, NDT, F], BF)
        wv_sb = wpool.tile([P, NDT, F], BF)
        nc.sync.dma_start(wq_sb[:], wqT_r)
        nc.sync.dma_start(wk_sb[:], wkT_r)
        nc.sync.dma_start(wv_sb[:], wvT_r)
        wo_sb = big.tile([P, NDT, D], BF)
        nc.sync.dma_start(wo_sb[:], woT_r)

        qT_sb = big.tile([P, HPC, S], BF)   # [dh, head, q]
        kT_sb = big.tile([P, HPC, S], BF)   # [dh, head, p]
        v_sb = big.tile([P, HPC, S // P, DH], BF)  # [p_in, head, p_out, h]

        # ================= phase 1: Q/K/V projections ==================
        for pt_i in range(NQT):
            sl = bass.ts(pt_i, QT)
            xt = xpool.tile([P, NDT, QT], BF, tag="xt")
            nc.sync.dma_start(xt[:], xT_r[:, :, sl])
            for h in range(HPC):
                hs = bass.ts(h, DH)
                psq = psum512.tile([P, QT], F32, tag="mm512")
                psk = psum512.tile([P, QT], F32, tag="mm512")
                for dt_i in range(NDT):
                    nc.tensor.matmul(psq[:], wq_sb[:, dt_i, hs], xt[:, dt_i],
                                     start=(dt_i == 0), stop=(dt_i == NDT - 1))
                for dt_i in range(NDT):
                    nc.tensor.matmul(psk[:], wk_sb[:, dt_i, hs], xt[:, dt_i],
                                     start=(dt_i == 0), stop=(dt_i == NDT - 1))
                nc.vector.tensor_copy(qT_sb[:, h, sl], psq[:])
                nc.vector.tensor_copy(kT_sb[:, h, sl], psk[:])
            for sub in range(QT // P):
                p_out = pt_i * (QT // P) + sub
                psv = psum512.tile([P, F], F32, tag="mm512")
                for dt_i in range(NDT):
                    nc.tensor.matmul(psv[:], xt[:, dt_i, bass.ts(sub, P)],
                                     wv_sb[:, dt_i],
                                     start=(dt_i == 0), stop=(dt_i == NDT - 1))
                for h in range(HPC):
                    nc.vector.tensor_copy(v_sb[:, h, p_out, :],
                                          psv[:, bass.ts(h, DH)])

        # ================= phase 2: causal attention ===================
        for h in range(HPC):
            for qi in range(NQT):
                qsl = bass.ts(qi, QT)
                zps = psum512.tile([P, QT], F32, tag="mm512", name="zps")
                dps = psum512.tile([1, QT], F32, tag="mm512", name="dps")
                npt = 4 * qi + 4  # p tiles below/at diagonal
                for pp in range(0, npt, 2):
                    sps = psum1024.tile([P, 2 * QT], F32, tag="mm1024",
                                         name="sps")
                    for u in range(2):
                        pi = pp + u
                        nc.tensor.matmul(
                            sps[:, bass.ts(u, QT)],
                            kT_sb[:, h, bass.ts(pi, P)],
                            qT_sb[:, h, qsl], start=True, stop=True)
                    pt = ptpool.tile([P, 2 * QT], BF, tag="pt")
                    nc.scalar.activation(pt[:], sps[:], Exp,
                                         scale=INV_SQRT_DH)
                    for u in range(2):
                        pi = pp + u
                        off = pi * P - qi * QT
                        if off >= 0:  # diagonal block -> apply causal mask
                            nc.vector.tensor_tensor(
                                pt[:, bass.ts(u, QT)], pt[:, bass.ts(u, QT)],
                                mask_sb[:, 384 - off: 384 - off + QT], mult)
                    for u in range(2):
                        pi = pp + u
                        nc.tensor.matmul(zps[:], v_sb[:, h, pi, :],
                                         pt[:, bass.ts(u, QT)],
                                         start=(pi == 0), stop=(pi == npt - 1))
                        nc.tensor.matmul(dps[:], ones_sb[:],
                                         pt[:, bass.ts(u, QT)],
                                         start=(pi == 0), stop=(pi == npt - 1))
                # softmax denominator -> reciprocal -> broadcast -> scale z
                recip = small.tile([1, QT], F32, tag="recip")
                nc.vector.reciprocal(recip[:], dps[:])
                bcast = small.tile([P, QT], F32, tag="bcast")
                nc.gpsimd.partition_broadcast(bcast[:], recip[:])
                zsb = small.tile([P, QT], BF, tag="zsb")
                nc.vector.tensor_tensor(zsb[:], zps[:], bcast[:], mult)
                nc.sync.dma_start(
                    z_send[qi * F + h * DH: qi * F + (h + 1) * DH, :], zsb[:])

        # ================= phase 3: AllToAll + out-proj ================
        nc.gpsimd.collective_compute(
            "AllToAll", mybir.AluOpType.bypass,
            replica_groups=[list(range(NCORES))],
            ins=[z_send[:]], outs=[z_recv[:]],
        )
        for qs in range(SQ // P):
            zt = ztpool.tile([P, NDT, P], BF, tag="zt")
            nc.sync.dma_start(zt[:], zr_r[:, :, bass.ts(qs, P)])
            for dt_i in range(D // QT):
                pso = psum512.tile([P, QT], F32, tag="mm512", name="pso")
                for ft in range(NDT):
                    nc.tensor.matmul(pso[:], zt[:, ft],
                                     wo_sb[:, ft, bass.ts(dt_i, QT)],
                                     start=(ft == 0), stop=(ft == NDT - 1))
                osb = opool.tile([P, QT], F32, tag="osb")
                nc.vector.tensor_copy(osb[:], pso[:])
                nc.sync.dma_start(out[bass.ts(qs, P), bass.ts(dt_i, QT)],
                                  osb[:])


_EXEC = None


def _get_exec():
    """Build (once) a non-donating jitted shard_map executor for the NEFF,
    so it can be invoked repeatedly for timing."""
    global _EXEC, _CACHED_NC
    if _EXEC is not None:
        return _EXEC
    if _CACHED_NC is None:
        _CACHED_NC = build()
    nc = _CACHED_NC
    import jax
    from jax.sharding import Mesh, PartitionSpec
    from jax.experimental.shard_map import shard_map
    from concourse.bass2jax import (_bass_exec_p, install_neuronx_cc_hook,
                                    partition_id_tensor)

    install_neuronx_cc_hook()
    partition_name = (nc.partition_id_tensor.name
                      if nc.partition_id_tensor else None)
    in_names, out_names, out_avals, zero_outs = [], [], [], []
    for alloc in nc.m.functions[0].allocations:
        if not isinstance(alloc, mybir.MemoryLocationSet):
            continue
        name = alloc.memorylocations[0].name
        if alloc.kind == "ExternalInput":
            if name != partition_name:
                in_names.append(name)
        elif alloc.kind == "ExternalOutput":
            out_names.append(name)
            shape = tuple(alloc.tensor_shape)
            dtype = mybir.dt.np(alloc.dtype)
            out_avals.append(jax.core.ShapedArray(shape, dtype))
            zero_outs.append(np.zeros(shape, dtype))
    n_params = len(in_names)
    in_names = in_names + out_names
    if partition_name is not None:
        in_names.append(partition_name)

    def _bd(*args):
        operands = list(args)
        if partition_name is not None:
            operands.append(partition_id_tensor())
        outs = _bass_exec_p.bind(
            *operands, out_avals=tuple(out_avals), in_names=tuple(in_names),
            out_names=tuple(out_names), lowering_input_output_aliases=(),
            sim_require_finite=True, sim_require_nnan=True, nc=nc)
        return tuple(outs)

    devices = jax.devices()[:NCORES]
    mesh = Mesh(np.asarray(devices), ("core",))
    nin = n_params + len(out_names)
    donate = tuple(range(n_params, nin))
    sharded = jax.jit(
        shard_map(_bd, mesh=mesh, in_specs=(PartitionSpec("core"),) * nin,
                  out_specs=(PartitionSpec("core"),) * len(out_names),
                  check_rep=False),
        donate_argnums=donate, keep_unused=True)
    from jax.sharding import NamedSharding
    zshard = NamedSharding(mesh, PartitionSpec("core"))

    def _mk_zeros():
        import jax.numpy as jnp
        return tuple(jnp.zeros((NCORES * z.shape[0], *z.shape[1:]), z.dtype)
                     for z in zero_outs)

    mk_zeros = jax.jit(_mk_zeros,
                       out_shardings=tuple(zshard for _ in zero_outs))
    _EXEC = (sharded, in_names[:n_params], out_names, out_avals, mk_zeros)
    return _EXEC


def _concat_inputs(in_maps):
    sharded, in_names, out_names, out_avals, mk_zeros = _get_exec()
    return [
        np.concatenate([np.asarray(in_maps[c][k]) for c in range(NCORES)],
                       axis=0) for k in in_names]


def _prep_inputs(x, W_K, W_Q, W_V, W_O):
    bf = ml_dtypes.bfloat16
    x2 = np.asarray(x, np.float32).reshape(S, D)
    xT = np.ascontiguousarray(x2.T).astype(bf)
    woT = np.ascontiguousarray(np.asarray(W_O, np.float32).T).astype(bf)
    in_maps = []
    for c in range(NCORES):
        m = {"xT": xT, "woT": woT}
        for name, W in (("wqT", W_Q), ("wkT", W_K), ("wvT", W_V)):
            w = np.asarray(W[2 * c: 2 * c + 2], np.float32)  # [2,128,2048]
            m[name] = np.ascontiguousarray(
                w.transpose(2, 0, 1).reshape(D, F)).astype(bf)
        in_maps.append(m)
    return in_maps


def run_dist(x, W_K, W_Q, W_V, W_O, time_it=False):
    """Run the distributed kernel. Returns (full_output, exec_ns_estimate).

    exec_ns_estimate (when time_it) is measured by slope: launch N back-to-back
    executions into the device queues and block once; the marginal per-call
    wall time amortizes the ~80 ms axon dispatch round-trip away.
    """
    import jax
    import time

    in_maps = _prep_inputs(x, W_K, W_Q, W_V, W_O)
    sharded, in_names, out_names, out_avals, mk_zeros = _get_exec()
    args_np = _concat_inputs(in_maps)
    args = [jax.device_put(a) for a in args_np]
    outs = sharded(*args, *mk_zeros())
    jax.block_until_ready(outs)
    full = np.asarray(outs[0]).reshape(1, S, D).astype(np.float32)

    exec_ns = None
    if time_it:
        def launch(n):
            t0 = time.perf_counter()
            rs = None
            for _ in range(n):
                rs = sharded(*args, *mk_zeros())
            jax.block_until_ready(rs)
            return time.perf_counter() - t0

        launch(3)  # warm
        deltas = []
        for _ in range(3):
            t_small = launch(2)
            t_big = launch(18)
            deltas.append((t_big - t_small) / 16)
        exec_ns = int(min(deltas) * 1e9)
    return full, exec_ns


def kernel(x, W_K, W_Q, W_V, W_O):
    full, _ = run_dist(x, W_K, W_Q, W_V, W_O)
    return full


# revision 44
# speedup vs baseline: 33.2670x; 33.2670x over previous
"""Distributed causal multi-head attention for Trainium2 (8 NeuronCores).

Problem: x:(1,4096,2048), W_{K,Q,V}:(16,128,2048), W_O:(2048,2048)
  k/q/v = einsum('ihd,bpd->biph'), scores=q@k^T causal-masked softmax, z=attn@v,
  out = einsum('df,bqf->bqd', W_O, z_flat)

Sharding: tensor-parallel over heads. Core c owns heads {2c, 2c+1}:
  - computes Q^T/K^T ([dh,seq] layout) and V ([seq,dh]) for its 2 heads
  - attention entirely on-core, scores kept transposed [key_pos, q] so the
    softmax denominator is a ones-matmul and AV needs no transposes
  - normalized z^T [f=head*128+h, q] is AllToAll'd so core c ends up with
    z^T[all 2048 f, its 512 q rows]; it then computes out rows [512c:512c+512]
    with the full W_O^T. Host concatenates the 8 row-blocks.

All matmul operands are bf16 (1 cycle/row on PE vs 4 for fp32); accumulation
fp32 in PSUM. exp/softmax stats in fp32.
"""

import os
import sys

import numpy as np

for _p in ("/opt/trn_rl_repo", "/root/.axon_site/_ro/trn_rl_repo"):
    if os.path.isdir(_p) and _p not in sys.path:
        sys.path.insert(0, _p)

import ml_dtypes  # noqa: E402

import concourse.bass as bass  # noqa: E402
import concourse.mybir as mybir  # noqa: E402
import concourse.tile as tile  # noqa: E402
from concourse import bacc  # noqa: E402
from concourse.bass_utils import run_bass_kernel_spmd  # noqa: E402

P = 128          # partitions
S = 4096         # sequence
D = 2048         # d_model
DH = 128         # head dim
NCORES = 8
HPC = 2          # heads per core
QT = 512         # q tile (matmul free dim)
NQT = S // QT    # 8 q tiles
SQ = S // NCORES # 512 out rows per core
NDT = D // P     # 16 d-model tiles
F = HPC * DH     # 256 local z rows
BF = mybir.dt.bfloat16
F32 = mybir.dt.float32
INV_SQRT_DH = 1.0 / float(np.sqrt(DH))

_CACHED_NC = None

# tunables (A/B-tested via time_twin(cfg=...))
DEFAULT_CFG = dict(
    out_bf16=True,     # write out as bf16 (host upcasts)
    wo_late=True,      # emit wo load after projections (off the x critical path)
    denom_dve=True,    # accumulate softmax denominator on VectorE, not PE
    a2a_split=True,    # two per-head AllToAlls; first overlaps head-1 attention
    x_split=2,         # DMAs per x pos-tile (stream count)
    xt_bufs=4,         # buffers per x stream
    wo_stream=True,    # stream wo per d-tile in out-proj (frees SBUF)
)


def build(variant="full", twin=False, cfg=None):
    """twin=True builds a timing twin: identical compute graph but inputs are
    internal (uninitialized) DRAM tensors and the external output is tiny, so
    per-execution host->device transfer (which under axon hides device time)
    is negligible and launch-slope timing measures the true NEFF duration."""
    cfg = {**DEFAULT_CFG, **(cfg or {})}
    nc = bacc.Bacc("TRN2", target_bir_lowering=False, debug=False,
                   num_devices=NCORES)

    ODT = BF if cfg["out_bf16"] else F32
    ikind = "Internal" if twin else "ExternalInput"
    # all inputs host-repacked so every DMA reads >=8KB contiguous runs
    xp = nc.dram_tensor("xp", [NQT, P, NDT, QT], BF, kind=ikind).ap()
    wqT = nc.dram_tensor("wqT", [P, NDT, F], BF, kind=ikind).ap()
    wkT = nc.dram_tensor("wkT", [P, NDT, F], BF, kind=ikind).ap()
    wvT = nc.dram_tensor("wvT", [P, NDT, F], BF, kind=ikind).ap()
    woT = nc.dram_tensor("woT", [D // QT, P, NDT, QT], BF, kind=ikind).ap()
    if twin:
        out = nc.dram_tensor("out_fake", [SQ, D], ODT).ap()
        out_ext = nc.dram_tensor("out", [P, 64], F32,
                                 kind="ExternalOutput").ap()
    else:
        out = nc.dram_tensor("out", [SQ, D], ODT, kind="ExternalOutput").ap()
        out_ext = None
    # AllToAll bounce buffers. Row r = dest-chunk r//256, f-row r%256.
    if cfg["a2a_split"]:
        z_send = [nc.dram_tensor(f"z_send{h}", [S // 4, QT], BF).ap()
                  for h in range(HPC)]
        z_recv = [nc.dram_tensor(f"z_recv{h}", [S // 4, QT], BF).ap()
                  for h in range(HPC)]
    else:
        z_send = nc.dram_tensor("z_send", [S // 2, QT], BF).ap()
        z_recv = nc.dram_tensor("z_recv", [S // 2, QT], BF).ap()

    xT_r = xp
    wqT_r = wqT
    wkT_r = wkT
    wvT_r = wvT
    woT_r = woT
    if cfg["a2a_split"]:
        zr_r = [z.rearrange("(o i) q -> i o q", i=P) for z in z_recv]
    else:
        zr_r = z_recv.rearrange("(o i) q -> i o q", i=P)   # [128,16,512]

    with tile.TileContext(nc) as tc:
        _body(tc, nc, xT_r, wqT_r, wkT_r, wvT_r, woT_r, z_send, z_recv, zr_r,
              out, variant, cfg)
        if twin:
            with tc.tile_pool(name="tw", bufs=1) as twp:
                d = twp.tile([P, 64], F32)
                nc.gpsimd.memset(d[:], 0.0)
                nc.sync.dma_start(out_ext, d[:])

    nc.compile()
    return nc


def _body(tc, nc, xT_r, wqT_r, wkT_r, wvT_r, woT_r, z_send, z_recv, zr_r,
          out, variant="full", cfg=None):
    cfg = {**DEFAULT_CFG, **(cfg or {})}
    mult = mybir.AluOpType.mult
    Exp = mybir.ActivationFunctionType.Exp

    with (
        tc.tile_pool(name="const", bufs=1) as const,
        tc.tile_pool(name="w", bufs=1) as wpool,
        tc.tile_pool(name="big", bufs=1) as big,
        tc.tile_pool(name="xt", bufs=2) as xpool,
        tc.tile_pool(name="pt", bufs=3) as ptpool,
        tc.tile_pool(name="small", bufs=2) as small,
        tc.tile_pool(name="zt", bufs=2) as ztpool,
        tc.tile_pool(name="osb", bufs=2) as opool,
        tc.tile_pool(name="psum512", bufs=4, space="PSUM") as psum512,
        tc.tile_pool(name="psum1024", bufs=2, space="PSUM") as psum1024,
    ):
        # ---- constants -------------------------------------------------
        ones_sb = const.tile([P, 1], BF)
        nc.gpsimd.memset(ones_sb[:], 1.0)
        # sliding causal mask: mask[p, j] = 1.0 if j - p - 384 >= 0 else 0
        # slice [384-off : 896-off] gives keep iff (qlocal - p - off) >= 0
        mask_sb = const.tile([P, 384 + QT], BF)
        nc.gpsimd.memset(mask_sb[:], 1.0)
        nc.gpsimd.affine_select(
            out=mask_sb[:], in_=mask_sb[:],
            compare_op=mybir.AluOpType.is_ge,
            fill=0.0, base=-384,
            pattern=[[1, 384 + QT]], channel_multiplier=-1,
        )

        # ---- persistent SBUF tensors ----------------------------------
        wq_sb = wpool.tile([P, NDT, F], BF)
        wk_sb = wpool.tile([P, NDT, F], BF)
        wv_sb = wpool.tile([P, NDT, F], BF)
        nodma = (variant == "compute")
        if not nodma:
            nc.sync.dma_start(wq_sb[:], wqT_r)
            nc.sync.dma_start(wk_sb[:], wkT_r)
            nc.sync.dma_start(wv_sb[:], wvT_r)
        else:
            nc.gpsimd.memset(wq_sb[:, 0, 0:4], 0.01)
            nc.gpsimd.memset(wk_sb[:, 0, 0:4], 0.01)
            nc.gpsimd.memset(wv_sb[:, 0, 0:4], 0.01)
        if cfg["wo_stream"]:
            wo_sb = None
        else:
            wo_sb = big.tile([P, D // QT, NDT, QT], BF)
            if not cfg["wo_late"] and not nodma:
                for g in range(D // QT):
                    nc.sync.dma_start(wo_sb[:, g], woT_r[g])
            if nodma:
                nc.gpsimd.memset(wo_sb[:, 0, 0, 0:4], 0.01)

        qT_sb = big.tile([P, HPC, S], BF)   # [dh, head, q]
        kT_sb = big.tile([P, HPC, S], BF)   # [dh, head, p]
        v_sb = big.tile([P, HPC, S // P, DH], BF)  # [p_in, head, p_out, h]

        if variant == "dmaonly":
            dmy = small.tile([P, QT], BF, tag="zsb")
            nc.gpsimd.memset(dmy[:], 0.25)
            for pt_i in range(NQT):
                nsp = cfg["x_split"]
                per = NDT // nsp
                for g in range(nsp):
                    t = xpool.tile([P, per, QT], BF, tag=f"xt{g}",
                                   bufs=cfg["xt_bufs"])
                    nc.sync.dma_start(
                        t[:], xT_r[pt_i, :, g * per:(g + 1) * per])
            for qi in range(NQT):
                for h in range(HPC):
                    if cfg["a2a_split"]:
                        nc.sync.dma_start(z_send[h][bass.ts(qi, DH), :],
                                          dmy[:])
                    else:
                        nc.sync.dma_start(
                            z_send[qi * F + h * DH: qi * F + (h + 1) * DH, :],
                            dmy[:])
            if cfg["a2a_split"]:
                for h in range(HPC):
                    nc.gpsimd.collective_compute(
                        "AllToAll", mybir.AluOpType.bypass,
                        replica_groups=[list(range(NCORES))],
                        ins=[z_send[h][:]], outs=[z_recv[h][:]],
                    )
                for hh in range(HPC):
                    zth = ztpool.tile([P, NQT, QT], BF, tag=f"ztf{hh}",
                                      bufs=1)
                    nc.sync.dma_start(zth[:], zr_r[hh])
            else:
                nc.gpsimd.collective_compute(
                    "AllToAll", mybir.AluOpType.bypass,
                    replica_groups=[list(range(NCORES))],
                    ins=[z_send[:]], outs=[z_recv[:]],
                )
                zt = ztpool.tile([P, NDT, QT], BF, tag="zta", bufs=1)
                nc.sync.dma_start(zt[:], zr_r)
            for g in range(D // QT):
                wt = ztpool.tile([P, NDT, QT], BF, tag="wos", bufs=2)
                nc.sync.dma_start(wt[:], woT_r[g])
            dmo = opool.tile([P, QT], BF if cfg["out_bf16"] else F32,
                             tag="osb")
            nc.vector.tensor_copy(dmo[:], dmy[:])
            for qs in range(SQ // P):
                for dt_i in range(D // QT):
                    nc.sync.dma_start(
                        out[bass.ts(qs, P), bass.ts(dt_i, QT)], dmo[:])
            return

        # ================= phase 1: Q/K/V projections ==================
        for pt_i in range(NQT):
            sl = bass.ts(pt_i, QT)
            nsp = cfg["x_split"]
            per = NDT // nsp
            xparts = []
            for g in range(nsp):
                t = xpool.tile([P, per, QT], BF, tag=f"xt{g}",
                               bufs=cfg["xt_bufs"])
                if not nodma:
                    nc.sync.dma_start(
                        t[:], xT_r[pt_i, :, g * per:(g + 1) * per])
                else:
                    nc.gpsimd.memset(t[:, 0, 0:4], 0.01)
                xparts.append(t)

            def xt(dt_i, xparts=xparts, per=per):
                return xparts[dt_i // per][:, dt_i % per]

            for h in range(HPC):
                hs = bass.ts(h, DH)
                psq = psum512.tile([P, QT], F32, tag="mm512")
                psk = psum512.tile([P, QT], F32, tag="mm512")
                for dt_i in range(NDT):
                    nc.tensor.matmul(psq[:], wq_sb[:, dt_i, hs], xt(dt_i),
                                     start=(dt_i == 0), stop=(dt_i == NDT - 1))
                for dt_i in range(NDT):
                    nc.tensor.matmul(psk[:], wk_sb[:, dt_i, hs], xt(dt_i),
                                     start=(dt_i == 0), stop=(dt_i == NDT - 1))
                nc.vector.tensor_copy(qT_sb[:, h, sl], psq[:])
                nc.vector.tensor_copy(kT_sb[:, h, sl], psk[:])
            for sub in range(QT // P):
                p_out = pt_i * (QT // P) + sub
                psv = psum512.tile([P, F], F32, tag="mm512")
                for dt_i in range(NDT):
                    nc.tensor.matmul(psv[:], xt(dt_i)[:, bass.ts(sub, P)],
                                     wv_sb[:, dt_i],
                                     start=(dt_i == 0), stop=(dt_i == NDT - 1))
                for h in range(HPC):
                    nc.vector.tensor_copy(v_sb[:, h, p_out, :],
                                          psv[:, bass.ts(h, DH)])

        if variant == "proj":
            dummy = opool.tile([P, QT], F32, tag="osb")
            nc.vector.tensor_copy(dummy[:], qT_sb[:, 0, 0:QT])
            for qs in range(SQ // P):
                for dt_i in range(D // QT):
                    nc.sync.dma_start(
                        out[bass.ts(qs, P), bass.ts(dt_i, QT)], dummy[:])
            return

        # ================= phase 2: causal attention ===================
        if cfg["wo_late"] and not nodma and not cfg["wo_stream"]:
            for g in range(D // QT):
                nc.sync.dma_start(wo_sb[:, g], woT_r[g])
        for h in range(HPC):
            if cfg["a2a_split"] and h > 0 and variant != "attn":
                nc.gpsimd.collective_compute(
                    "AllToAll", mybir.AluOpType.bypass,
                    replica_groups=[list(range(NCORES))],
                    ins=[z_send[h - 1][:]], outs=[z_recv[h - 1][:]],
                )
            for qi in range(NQT):
                qsl = bass.ts(qi, QT)
                zps = psum512.tile([P, QT], F32, tag="mm512", name="zps")
                npt = 4 * qi + 4  # p tiles below/at diagonal
                if cfg["denom_dve"]:
                    dps = None
                    acc = small.tile([P, QT], F32, tag="acc", bufs=2)
                else:
                    dps = psum512.tile([1, QT], F32, tag="mm512", name="dps")
                    acc = None
                for pp in range(0, npt, 2):
                    sps = psum1024.tile([P, 2 * QT], F32, tag="mm1024",
                                         name="sps")
                    for u in range(2):
                        pi = pp + u
                        nc.tensor.matmul(
                            sps[:, bass.ts(u, QT)],
                            kT_sb[:, h, bass.ts(pi, P)],
                            qT_sb[:, h, qsl], start=True, stop=True)
                    pt = ptpool.tile([P, 2 * QT], BF, tag="pt")
                    nc.scalar.activation(pt[:], sps[:], Exp,
                                         scale=INV_SQRT_DH)
                    for u in range(2):
                        pi = pp + u
                        off = pi * P - qi * QT
                        if off >= 0:  # diagonal block -> apply causal mask
                            nc.vector.tensor_tensor(
                                pt[:, bass.ts(u, QT)], pt[:, bass.ts(u, QT)],
                                mask_sb[:, 384 - off: 384 - off + QT], mult)
                    if cfg["denom_dve"]:
                        if pp == 0:
                            nc.vector.tensor_tensor(
                                acc[:], pt[:, 0:QT], pt[:, QT:2 * QT],
                                mybir.AluOpType.add)
                        else:
                            nc.vector.tensor_tensor(
                                acc[:], acc[:], pt[:, 0:QT],
                                mybir.AluOpType.add)
                            nc.vector.tensor_tensor(
                                acc[:], acc[:], pt[:, QT:2 * QT],
                                mybir.AluOpType.add)
                    for u in range(2):
                        pi = pp + u
                        nc.tensor.matmul(zps[:], v_sb[:, h, pi, :],
                                         pt[:, bass.ts(u, QT)],
                                         start=(pi == 0), stop=(pi == npt - 1))
                        if not cfg["denom_dve"]:
                            nc.tensor.matmul(dps[:], ones_sb[:],
                                             pt[:, bass.ts(u, QT)],
                                             start=(pi == 0),
                                             stop=(pi == npt - 1))
                # softmax denominator -> reciprocal -> broadcast -> scale z
                if cfg["denom_dve"]:
                    accb = small.tile([P, QT], BF, tag="accb", bufs=2)
                    nc.vector.tensor_copy(accb[:], acc[:])
                    dps = psum512.tile([1, QT], F32, tag="mm512", name="dps")
                    nc.tensor.matmul(dps[:], ones_sb[:], accb[:],
                                     start=True, stop=True)
                recip = small.tile([1, QT], BF, tag="recip")
                with nc.allow_low_precision(reason="bf16 softmax recip"):
                    nc.vector.reciprocal(recip[:], dps[:])
                bcast = small.tile([P, QT], BF, tag="bcast")
                nc.gpsimd.partition_broadcast(bcast[:], recip[:])
                zsb = small.tile([P, QT], BF, tag="zsb")
                nc.vector.tensor_tensor(zsb[:], zps[:], bcast[:], mult)
                if cfg["a2a_split"]:
                    nc.sync.dma_start(z_send[h][bass.ts(qi, DH), :], zsb[:])
                else:
                    nc.sync.dma_start(
                        z_send[qi * F + h * DH: qi * F + (h + 1) * DH, :],
                        zsb[:])

        if variant == "attn":
            dummy = opool.tile([P, QT], F32, tag="osb")
            nc.vector.tensor_copy(dummy[:], qT_sb[:, 0, 0:QT])
            for qs in range(SQ // P):
                for dt_i in range(D // QT):
                    nc.sync.dma_start(
                        out[bass.ts(qs, P), bass.ts(dt_i, QT)], dummy[:])
            return

        # ================= phase 3: AllToAll + out-proj ================
        if cfg["a2a_split"]:
            nc.gpsimd.collective_compute(
                "AllToAll", mybir.AluOpType.bypass,
                replica_groups=[list(range(NCORES))],
                ins=[z_send[HPC - 1][:]], outs=[z_recv[HPC - 1][:]],
            )
        elif variant == "nocc":
            nc.sync.dma_start(z_recv[:], z_send[:])
        else:
            nc.gpsimd.collective_compute(
                "AllToAll", mybir.AluOpType.bypass,
                replica_groups=[list(range(NCORES))],
                ins=[z_send[:]], outs=[z_recv[:]],
            )
        wo_stream_tiles = []
        if cfg["a2a_split"]:
            # one 1MB load per head of the whole gathered z (bigger DMAs win)
            zfull = []
            for hh in range(HPC):
                zth = ztpool.tile([P, NQT, QT], BF, tag=f"ztf{hh}", bufs=1)
                nc.sync.dma_start(zth[:], zr_r[hh])
                zfull.append(zth)
        else:
            zt_all = ztpool.tile([P, NDT, QT], BF, tag="zta", bufs=1)
            nc.sync.dma_start(zt_all[:], zr_r)

        def ztile(ft, qs):
            if cfg["a2a_split"]:
                return zfull[ft % 2][:, ft // 2, bass.ts(qs, P)]
            return zt_all[:, ft, bass.ts(qs, P)]

        # head-0 f-tiles (even) first: their A2A lands earlier
        ft_order = ([ft for ft in range(NDT) if ft % 2 == 0] +
                    [ft for ft in range(NDT) if ft % 2 == 1]
                    ) if cfg["a2a_split"] else list(range(NDT))
        for dt_i in range(D // QT):
            if cfg["wo_stream"]:
                wt = ztpool.tile([P, NDT, QT], BF, tag="wos", bufs=2)
                if variant == "compute":
                    nc.gpsimd.memset(wt[:, 0, 0:4], 0.01)
                else:
                    nc.sync.dma_start(wt[:], woT_r[dt_i])

                def wo_rhs(ft, wt=wt):
                    return wt[:, ft]
            else:
                def wo_rhs(ft, dt_i=dt_i):
                    return wo_sb[:, dt_i, ft]
            for qs in range(SQ // P):
                pso = psum512.tile([P, QT], F32, tag="mm512", name="pso")
                for k, ft in enumerate(ft_order):
                    nc.tensor.matmul(pso[:], ztile(ft, qs), wo_rhs(ft),
                                     start=(k == 0), stop=(k == NDT - 1))
                osb = opool.tile([P, QT],
                                 BF if cfg["out_bf16"] else F32, tag="osb")
                nc.vector.tensor_copy(osb[:], pso[:])
                nc.sync.dma_start(out[bass.ts(qs, P), bass.ts(dt_i, QT)],
                                  osb[:])


_EXEC = None
_EXEC_CACHE = {}


def get_exec(nc, key=None):
    """Build (once per nc) a jitted shard_map executor for the NEFF."""
    if key is not None and key in _EXEC_CACHE:
        return _EXEC_CACHE[key]
    import jax
    from jax.sharding import Mesh, PartitionSpec, NamedSharding
    from jax.experimental.shard_map import shard_map
    from concourse.bass2jax import (_bass_exec_p, install_neuronx_cc_hook,
                                    partition_id_tensor)

    install_neuronx_cc_hook()
    partition_name = (nc.partition_id_tensor.name
                      if nc.partition_id_tensor else None)
    in_names, out_names, out_avals, zero_outs = [], [], [], []
    for alloc in nc.m.functions[0].allocations:
        if not isinstance(alloc, mybir.MemoryLocationSet):
            continue
        name = alloc.memorylocations[0].name
        if alloc.kind == "ExternalInput":
            if name != partition_name:
                in_names.append(name)
        elif alloc.kind == "ExternalOutput":
            out_names.append(name)
            shape = tuple(alloc.tensor_shape)
            dtype = mybir.dt.np(alloc.dtype)
            out_avals.append(jax.core.ShapedArray(shape, dtype))
            zero_outs.append(np.zeros(shape, dtype))
    n_params = len(in_names)
    in_names = in_names + out_names
    if partition_name is not None:
        in_names.append(partition_name)

    def _bd(*args):
        operands = list(args)
        if partition_name is not None:
            operands.append(partition_id_tensor())
        outs = _bass_exec_p.bind(
            *operands, out_avals=tuple(out_avals), in_names=tuple(in_names),
            out_names=tuple(out_names), lowering_input_output_aliases=(),
            sim_require_finite=True, sim_require_nnan=True, nc=nc)
        return tuple(outs)

    devices = jax.devices()[:NCORES]
    mesh = Mesh(np.asarray(devices), ("core",))
    nin = n_params + len(out_names)
    donate = tuple(range(n_params, nin))
    sharded = jax.jit(
        shard_map(_bd, mesh=mesh, in_specs=(PartitionSpec("core"),) * nin,
                  out_specs=(PartitionSpec("core"),) * len(out_names),
                  check_rep=False),
        donate_argnums=donate, keep_unused=True)
    zshard = NamedSharding(mesh, PartitionSpec("core"))

    def _mk_zeros():
        import jax.numpy as jnp
        return tuple(jnp.zeros((NCORES * z.shape[0], *z.shape[1:]), z.dtype)
                     for z in zero_outs)

    mk_zeros = jax.jit(_mk_zeros,
                       out_shardings=tuple(zshard for _ in zero_outs))
    res = (sharded, in_names[:n_params], out_names, out_avals, mk_zeros)
    if key is not None:
        _EXEC_CACHE[key] = res
    return res


def _get_exec():
    global _CACHED_NC
    if _CACHED_NC is None:
        _CACHED_NC = build()
    return get_exec(_CACHED_NC, key="main")


def time_exec(sharded, args, mk_zeros, reps=(2, 17), rounds=3):
    """Marginal per-execution wall time via launch-count slope."""
    import jax
    import time

    def launch(n):
        t0 = time.perf_counter()
        rs = None
        for _ in range(n):
            rs = sharded(*args, *mk_zeros())
        jax.block_until_ready(rs)
        return time.perf_counter() - t0

    launch(3)
    n0, n1 = reps
    return min((launch(n1) - launch(n0)) / (n1 - n0) for _ in range(rounds))


def time_many(specs, rounds=6, reps=(3, 43)):
    """Compile all (variant, cfg) specs, then interleave launches.
    Takes min wall time per rep-count across rounds FIRST, then the slope
    (robust to one-off dispatch stalls). Returns {name: ms}."""
    execs = {}
    for name, variant, cfg in specs:
        key = ("twin", variant, tuple(sorted((cfg or {}).items())))
        if key not in _EXEC_CACHE:
            get_exec(build(variant, twin=True, cfg=cfg), key=key)
        sharded, in_names, _, _, mk_zeros = _EXEC_CACHE[key]
        execs[name] = (sharded, mk_zeros)
        time_exec(sharded, [], mk_zeros, reps=(1, 2), rounds=1)  # warm/load
    import time as _t
    import jax
    n0, n1 = reps
    best = {name: {n0: float("inf"), n1: float("inf")} for name in execs}
    for _ in range(rounds):
        for name, (sharded, mk_zeros) in execs.items():
            for n in (n0, n1):
                t0 = _t.perf_counter()
                rs = None
                for _ in range(n):
                    rs = sharded(*mk_zeros())
                jax.block_until_ready(rs)
                dt = _t.perf_counter() - t0
                best[name][n] = min(best[name][n], dt)
    return {name: (b[n1] - b[n0]) / (n1 - n0) * 1e3
            for name, b in best.items()}


def time_twin(variant="full", cfg=None):
    """True NEFF exec time (ns): timing twin with no input transfer."""
    key = ("twin", variant, tuple(sorted((cfg or {}).items())))
    if key not in _EXEC_CACHE:
        nc = build(variant, twin=True, cfg=cfg)
        get_exec(nc, key=key)
    sharded, in_names, out_names, out_avals, mk_zeros = _EXEC_CACHE[key]
    assert not in_names, in_names
    return int(time_exec(sharded, [], mk_zeros, reps=(2, 102), rounds=3) * 1e9)


def _concat_inputs(in_maps):
    sharded, in_names, out_names, out_avals, mk_zeros = _get_exec()
    return [
        np.concatenate([np.asarray(in_maps[c][k]) for c in range(NCORES)],
                       axis=0) for k in in_names]


def _prep_inputs(x, W_K, W_Q, W_V, W_O):
    bf = ml_dtypes.bfloat16
    x2 = np.asarray(x, np.float32).reshape(S, D)
    xT = x2.T  # [d, s]
    # xp[pt, di, dt, s] = xT[dt*128+di, pt*512+s] -> 16KB contiguous/partition
    xp = np.ascontiguousarray(
        xT.reshape(NDT, P, NQT, QT).transpose(2, 1, 0, 3)).astype(bf)
    woT = np.asarray(W_O, np.float32).T  # [f, d]
    wop = np.ascontiguousarray(
        woT.reshape(NDT, P, D // QT, QT).transpose(2, 1, 0, 3)).astype(bf)
    in_maps = []
    for c in range(NCORES):
        m = {"xp": xp, "woT": wop}
        for name, W in (("wqT", W_Q), ("wkT", W_K), ("wvT", W_V)):
            w = np.asarray(W[2 * c: 2 * c + 2], np.float32)  # [2,128,2048]
            wt = w.transpose(2, 0, 1).reshape(D, F)  # [d, f]
            m[name] = np.ascontiguousarray(
                wt.reshape(NDT, P, F).transpose(1, 0, 2)).astype(bf)
        in_maps.append(m)
    return in_maps


def run_dist(x, W_K, W_Q, W_V, W_O, time_it=False):
    """Run the distributed kernel. Returns (full_output, exec_ns_estimate).

    exec_ns_estimate (when time_it) is measured by slope: launch N back-to-back
    executions into the device queues and block once; the marginal per-call
    wall time amortizes the ~80 ms axon dispatch round-trip away.
    """
    import jax
    import time

    in_maps = _prep_inputs(x, W_K, W_Q, W_V, W_O)
    sharded, in_names, out_names, out_avals, mk_zeros = _get_exec()
    args_np = _concat_inputs(in_maps)
    args = [jax.device_put(a) for a in args_np]
    outs = sharded(*args, *mk_zeros())
    jax.block_until_ready(outs)
    full = np.asarray(outs[0]).reshape(1, S, D).astype(np.float32)

    exec_ns = None
    if time_it:
        def launch(n):
            t0 = time.perf_counter()
            rs = None
            for _ in range(n):
                rs = sharded(*args, *mk_zeros())
            jax.block_until_ready(rs)
            return time.perf_counter() - t0

        launch(3)  # warm
        deltas = []
        for _ in range(3):
            t_small = launch(2)
            t_big = launch(18)
            deltas.append((t_big - t_small) / 16)
        exec_ns = int(min(deltas) * 1e9)
    return full, exec_ns


def kernel(x, W_K, W_Q, W_V, W_O):
    full, _ = run_dist(x, W_K, W_Q, W_V, W_O)
    return full
